# revision 28
# baseline (speedup 1.0000x reference)
"""Trainium2 Bass kernel for a dense transformer decoder block.

Reference computation (B=4, S=2048, D=768, H=12, DK=64, DF=3072):
    q,k,v = x@wq+bq, x@wk+bk, x@wv+bv          (per-head split, DK=64)
    attn  = softmax(mask(q k^T / 8))
    ctx   = attn @ v
    h     = LN(ctx@wo + bo + x; g1, be1)
    out   = LN(gelu_exact(h@w1 + b1)@w2 + b2 + h; g2, be2)

Sharding: pure data parallel, zero collectives. 8 cores = 4 batch elements
x 2 query groups of 1024 rows. Queries are interleaved at 256-row slot
granularity so the SPMD-uniform per-slot key extents (4, 8, 12, 16) x 128
cover both cores' causal needs with only 20 (vs 24) key-block units of
attention work per core; only the last 4 blocks of each slot's extent ever
need the data mask.
Core 2b+0: 256-row slots {0, 2, 4, 6} of batch b.
Core 2b+1: 256-row slots {1, 3, 5, 7} of batch b.
Every core runs the identical SPMD program; per-core behavior differs only
through input data (sliced/transposed/cast on the host).

Schedule: attention is ACT-(exp)-bound, so independent PE work is woven
between attention iterations to keep the tensor engine dense (and its HAM
clock warm): the sb2/sb3 K,V projections run under qb0 attention, and the
qb0 out-projection + LN1 + h-transposes run under qb1 attention.
"""

from contextlib import ExitStack

import numpy as np
import ml_dtypes

import concourse.bass as bass
import concourse.tile as tile
from concourse import bacc, mybir
from concourse.bass_utils import run_bass_kernel_spmd
from concourse.masks import make_identity

F32 = mybir.dt.float32
BF16 = mybir.dt.bfloat16
AF = mybir.ActivationFunctionType
OP = mybir.AluOpType
BF = ml_dtypes.bfloat16

B, S, D, H, DK = 4, 2048, 768, 12, 64
DF = 4 * D
EPS = 1e-5
P = 128
SQ = 1024            # query rows per core
HP = H // 2          # 6 head pairs
KB = S // P          # 16 key blocks
QB = 2               # query slot-pairs of 512 per core
QBS = 512
SLOT = 256           # attention query slot: 4 slots of 256 rows per core
EXTS = [4, 8, 12, 16]  # key-block extent per slot (block-causal skip)
# Interleaved slot assignment: core 2b+0 gets 256-row blocks {0,2,4,6} of
# batch b, core 2b+1 gets {1,3,5,7}. Slot s on either core then needs at
# most EXTS[s] key blocks, cutting SPMD-uniform attention volume from 24
# key-block-x-512q units to 20 per core, with only the last 4 blocks of
# each slot's extent ever needing the data mask.
DC = D // P          # 6 chunks of the model dim
DFC = DF // P        # 24 chunks of the FFN dim
QC = SQ // P         # 8 query chunks of 128
NH = 2               # 384-wide halves of D for PSUM-friendly matmul N
NHW = D // NH        # 384
SB = S // QBS        # 4 key column slabs

N_CORES = 8


def emit(ctx: ExitStack, tc: tile.TileContext, io: dict):
    nc = tc.nc

    xT, xqT, xres, maskT = io["xT"], io["xqT"], io["xres"], io["maskT"]
    wq, wk, wv, wo, w1, w2 = io["wq"], io["wk"], io["wv"], io["wo"], io["w1"], io["w2"]
    out = io["out"]

    # ---- constants ----------------------------------------------------
    const = ctx.enter_context(tc.tile_pool(name="const", bufs=1))
    ident = const.tile([P, P], BF16)
    make_identity(nc, ident)
    eps_t = const.tile([P, 1], F32)
    nc.vector.memset(eps_t, EPS)

    bqp = const.tile([P, HP], F32)
    nc.gpsimd.dma_start(out=bqp, in_=io["bqp"])
    bkp = const.tile([P, HP], F32)
    nc.gpsimd.dma_start(out=bkp, in_=io["bkp"])
    b1p = const.tile([P, DFC], F32)
    nc.gpsimd.dma_start(out=b1p, in_=io["b1p"])

    def brow(name):
        # [1, D] dram tensor broadcast-DMA'd across 128 partitions
        t = const.tile([P, D], F32, tag=name)
        a = io[name]
        src = bass.AP(tensor=a.tensor, offset=a.offset, ap=[[0, P]] + list(a.ap[1:]))
        nc.gpsimd.dma_start(out=t, in_=src)
        return t

    g1b, be1b, g2b, be2b, b2b = map(brow, ["g1r", "be1r", "g2r", "be2r", "b2r"])

    # ---- FFN-phase tensors: left stack, below attn_in so release order
    # stays LIFO (h/hT are written during the attention epilogue fillers)
    ffn = tc.alloc_tile_pool(name="ffn", bufs=1)
    h_sb = ffn.tile([P, QC, D], BF16)     # LN1 out (residual + FFN rhs)
    hT = ffn.tile([P, DC, SQ], BF16)
    ln_wk = tc.alloc_tile_pool(name="ln_wk", bufs=1)

    # ---- attention inputs (live through attention) --------------------
    attn_in = tc.alloc_tile_pool(name="attn_in", bufs=1)
    KT = attn_in.tile([P, HP, S], BF16)            # K^T, head pairs on partitions
    Vaug = attn_in.tile([P, KB, H, DK + 1], BF16)  # V + ones column per head
    QT = attn_in.tile([P, HP, SQ], BF16)
    # host-packed mask: for each slot s, its last 4 extent blocks
    # [EXTS[s]-4, EXTS[s]) -- the only blocks any core ever masks
    mTs = attn_in.tile([P, 4 * len(EXTS), SLOT], BF16)
    mr = maskT.rearrange("(m p) q -> p m q", p=P)
    nc.gpsimd.dma_start(out=mTs, in_=mr)
    nc.vector.memset(Vaug[:, :, :, DK : DK + 1], 1.0)

    # ---- post-attention inputs (right-side stack, phase-scoped) -------
    mid_ctx = tc.alloc_tile_pool(name="mid_ctx", bufs=1, side="right")
    ctxT = mid_ctx.tile([P, DC, SQ], BF16)

    kv_in = tc.alloc_tile_pool(name="kv_in", bufs=1, side="right")
    wk_sb = kv_in.tile([P, DC, D], BF16)
    wv_sb = kv_in.tile([P, DC, D], BF16)
    xT23 = kv_in.tile([P, DC, S // 2], BF16)
    xt01p = tc.alloc_tile_pool(name="xt01p", bufs=1, side="right")
    xT01 = xt01p.tile([P, DC, S // 2], BF16)
    xTr = xT.rearrange("(c p) s -> p c s", p=P)

    def xT_at(sb):
        t = xT01 if sb < 2 else xT23
        return t, (sb % 2) * QBS

    q_in = tc.alloc_tile_pool(name="q_in", bufs=1, side="right")
    wq_sb = q_in.tile([P, DC, D], BF16)
    xqT_sb = q_in.tile([P, DC, SQ], BF16)
    # split wq per head-pair column so q_unit(hp, 0) can start as soon as its
    # own slice + the first xqT slab land (instead of gating on the full 1.2MB)
    wqr = wq.rearrange("(c p) n -> p c n", p=P)
    for hp in range(HP):
        nc.sync.dma_start(out=wq_sb[:, :, hp * P : (hp + 1) * P],
                          in_=wqr[:, :, hp * P : (hp + 1) * P])
    xqr = xqT.rearrange("(c p) s -> p c s", p=P)
    for sb in range(2):
        nc.scalar.dma_start(out=xqT_sb[:, :, sb * QBS : (sb + 1) * QBS],
                            in_=xqr[:, :, sb * QBS : (sb + 1) * QBS])
    for sb in range(SB):
        dst = xT01 if sb < 2 else xT23
        nc.sync.dma_start(out=dst[:, :, (sb % 2) * QBS : (sb % 2 + 1) * QBS],
                          in_=xTr[:, :, sb * QBS : (sb + 1) * QBS])
    for c in range(DC):
        nc.scalar.dma_start(out=wk_sb[:, c, :],
                            in_=wk.rearrange("(c p) n -> p c n", p=P)[:, c, :])
    for c in range(DC):
        # separate queue from wk so K and V weight loads stream in parallel
        nc.gpsimd.dma_start(out=wv_sb[:, c, :],
                            in_=wv.rearrange("(c p) n -> p c n", p=P)[:, c, :])

    def ln_stats(wk_pool, src, mv_out):
        # DVE-only first half of LN: mean/var of fp32 src [128, 768] -> mv_out
        stats = wk_pool.tile([P, 3, 6], F32, tag="stats", bufs=3)
        for j in range(3):
            nc.vector.bn_stats(out=stats[:, j, :], in_=src[:, j * 256 : (j + 1) * 256])
        nc.vector.bn_aggr(out=mv_out, in_=stats)

    def ln_rstd_batch(mv_all, rstd_all, n):
        # one ACT Sqrt for n LN instances at once: Sqrt lives in a different
        # ACT table set than Exp, so batching keeps the table thrash off the
        # attention phase's exp-saturated ACT queue
        std_n = rstd_all  # in-place staging: sqrt then approx-reciprocal
        nc.scalar.activation(out=std_n[:, 0:n], in_=mv_all[:, 0:n, 1],
                             func=AF.Sqrt, bias=eps_t[:, 0:1])
        nc.vector.reciprocal_approx_fast(out=rstd_all[:, 0:n], in_=std_n[:, 0:n])

    def ln_apply(src, mv, rstd, gb, bb, dst):
        # second half of LN in two fused DVE ops:
        # t = (src - mu) * g;  dst = t * rstd + b
        nc.vector.scalar_tensor_tensor(out=src, in0=src, scalar=mv[:, 0:1],
                                       in1=gb, op0=OP.subtract, op1=OP.mult)
        nc.vector.scalar_tensor_tensor(out=dst, in0=src, scalar=rstd,
                                       in1=bb, op0=OP.mult, op1=OP.add)

    def layer_norm(wk_pool, src, gb, bb, dst):
        # full inline LN for the FFN epilogue (off the exp-critical phase)
        mv = wk_pool.tile([P, 1, 2], F32, tag="mv", bufs=3)
        ln_stats(wk_pool, src, mv[:, 0, :])
        rstd = wk_pool.tile([P, 1], F32, tag="rstd", bufs=3)
        ln_rstd_batch(mv, rstd, 1)
        ln_apply(src, mv[:, 0, :], rstd[:, 0:1], gb, bb, dst)

    proj_ps = tc.alloc_tile_pool(name="proj_ps", bufs=2, space="PSUM", side="right")
    with tc.tile_pool(name="sc_ps", bufs=2, space="PSUM") as sc_ps, \
         tc.tile_pool(name="cx_ps", bufs=1, space="PSUM") as cx_ps:
        # at_sb/nm_sb are allocated only once the q-projection inputs are
        # released -- their SBUF footprints must not overlap
        pools = {}

        # ---------- projection work units ----------
        def q_unit(hp, sb):
            ps = proj_ps.tile([P, QBS], F32, tag="proj")
            for c in range(DC):
                nc.tensor.matmul(
                    ps, lhsT=wq_sb[:, c, hp * P : (hp + 1) * P],
                    rhs=xqT_sb[:, c, sb * QBS : (sb + 1) * QBS],
                    start=(c == 0), stop=(c == DC - 1),
                )
            nc.scalar.activation(
                out=QT[:, hp, sb * QBS : (sb + 1) * QBS], in_=ps,
                func=AF.Identity, bias=bqp[:, hp : hp + 1],
            )

        def k_unit(hp, sb, on_act=True):
            xt, off = xT_at(sb)
            ps = proj_ps.tile([P, QBS], F32, tag="proj")
            for c in range(DC):
                nc.tensor.matmul(
                    ps, lhsT=wk_sb[:, c, hp * P : (hp + 1) * P],
                    rhs=xt[:, c, off : off + QBS],
                    start=(c == 0), stop=(c == DC - 1),
                )
            if on_act:
                nc.scalar.activation(
                    out=KT[:, hp, sb * QBS : (sb + 1) * QBS], in_=ps,
                    func=AF.Identity, bias=bkp[:, hp : hp + 1],
                )
            else:
                # inside the attention interleave ACT is the bottleneck chain
                nc.vector.tensor_scalar_add(
                    out=KT[:, hp, sb * QBS : (sb + 1) * QBS], in0=ps,
                    scalar1=bkp[:, hp : hp + 1],
                )

        def v_unit(kb, nh, on_act=True):
            xt, off = xT_at(kb // (QBS // P))
            kb_off = off // P + kb % (QBS // P)
            ps = proj_ps.tile([P, QBS], F32, tag="proj")
            psv = ps[:, 0:NHW]
            for c in range(DC):
                nc.tensor.matmul(
                    psv, lhsT=xt[:, c, kb_off * P : (kb_off + 1) * P],
                    rhs=wv_sb[:, c, nh * NHW : (nh + 1) * NHW],
                    start=(c == 0), stop=(c == DC - 1),
                )
            if on_act:
                nc.scalar.activation(
                    out=Vaug[:, kb, nh * 6 : (nh + 1) * 6, 0:DK],
                    in_=psv.rearrange("p (h d) -> p h d", d=DK),
                    func=AF.Copy,
                )
            else:
                nc.vector.tensor_copy(
                    out=Vaug[:, kb, nh * 6 : (nh + 1) * 6, 0:DK],
                    in_=psv.rearrange("p (h d) -> p h d", d=DK),
                )

        def kv_slab(sb, on_act=True):
            for hp in range(HP):
                k_unit(hp, sb, on_act)
            for j in range(QBS // P):
                for nh in range(NH):
                    v_unit(sb * (QBS // P) + j, nh, on_act)

        # ---------- attention iteration ----------
        pending = []

        def make_norm(cxs_e, cxs_o, den2, hp, qs):
            def go():
                # one reciprocal serves both heads: its cost scales with the
                # free size, not the partition count; dens are in (0, 2048] so
                # the ~51-ULP approx is safe and ~5x faster than the iterative
                # divide
                rec2 = pools['nm_sb'].tile([DK + 1, QBS], F32, tag="rec2", bufs=1)
                nc.vector.reciprocal_approx_fast(out=rec2, in_=den2)
                # partition_broadcast replicates the tile's physical partition
                # 0, so the head-odd reciprocal must move to its own base-0
                # tile first
                rec_o = pools['nm_sb'].tile([1, QBS], F32, tag="rec_o", bufs=1)
                nc.vector.tensor_copy(out=rec_o, in_=rec2[DK : DK + 1, :])
                for i, (cxs, pb) in enumerate(((cxs_e, 0), (cxs_o, DK))):
                    src_r = rec2[0:1, :] if i == 0 else rec_o[0:1, :]
                    den_b = pools['nm_sb'].tile([DK, QBS], F32, tag="den_b", bufs=1)
                    nc.gpsimd.partition_broadcast(den_b, src_r)
                    nc.vector.tensor_tensor(
                        out=ctxT[pb : pb + DK, hp, qs], in0=cxs[0:DK, :],
                        in1=den_b, op=OP.mult,
                    )
            return go

        def attn_iter(hp, sp, fill=None):
            # one slot-pair: slots (2*sp, 2*sp+1), 256 queries each; the
            # normalize machinery operates on the combined 512-query span
            qs = slice(sp * QBS, (sp + 1) * QBS)
            cxs_e = pools['nm_sb'].tile([DK + 1, QBS], F32, tag="cxs_e")
            cxs_o = pools['nm_sb'].tile([DK + 1, QBS], F32, tag="cxs_o")
            den2 = pools['nm_sb'].tile([DK + 1, QBS], F32, tag="den2")
            nc.vector.memset(den2, 1.0)
            npair = 0
            for si in range(2):
                s = 2 * sp + si
                ext = EXTS[s]
                sq = slice(s * SLOT, (s + 1) * SLOT)
                so = si * SLOT
                cx_e = cx_ps.tile([DK + 1, SLOT], F32, tag="cx_e")
                cx_o = cx_ps.tile([DK + 1, SLOT], F32, tag="cx_o")
                for gb in range(0, ext, 2):
                    pt = pools['at_sb'].tile([P, 2, 2, SLOT], BF16, tag="pt")
                    sc = sc_ps.tile([P, 2, 2, SLOT], F32, tag="sc")
                    for gi in range(2):
                        g = gb + gi
                        ks = slice(g * P, (g + 1) * P)
                        # the two heads of a pair hit disjoint PE row groups
                        # and run concurrently in the array
                        nc.tensor.matmul(sc[:, 0, gi, :], lhsT=KT[0:DK, hp, ks],
                                         rhs=QT[0:DK, hp, sq],
                                         start=True, stop=True)
                        nc.tensor.matmul(sc[:, 1, gi, :], lhsT=KT[DK:P, hp, ks],
                                         rhs=QT[DK:P, hp, sq],
                                         start=True, stop=True)
                    # one exp per g-pair (1024 elems) keeps the per-ACTIVATE
                    # 352-cycle overhead amortized despite the smaller slots
                    nc.scalar.activation(out=pt, in_=sc, func=AF.Exp,
                                         scale=1.0 / 8.0)
                    if gb >= ext - 4:
                        mi = 4 * s + gb - (ext - 4)
                        mq = mTs[:, mi : mi + 2, :]
                        for hh in range(2):
                            nc.vector.tensor_tensor(
                                out=pt[:, hh, :, :], in0=pt[:, hh, :, :],
                                in1=mq, op=OP.mult,
                            )
                    for gi in range(2):
                        g = gb + gi
                        nc.tensor.matmul(cx_e, lhsT=Vaug[:, g, 2 * hp, :],
                                         rhs=pt[:, 0, gi, :],
                                         start=(g == 0), stop=(g == ext - 1))
                        nc.tensor.matmul(cx_o, lhsT=Vaug[:, g, 2 * hp + 1, :],
                                         rhs=pt[:, 1, gi, :],
                                         start=(g == 0), stop=(g == ext - 1))
                    if si == 0 and gb == 2 and pending:
                        # previous iteration's normalize: emitted after this
                        # iteration's first blocks so the DVE reciprocal
                        # never delays the mask multiplies
                        pending.pop()()
                    # the PE is in-order: filler matmuls only absorb the
                    # exp-wait bubbles if woven BETWEEN key-block groups
                    if fill and (sp == 0 or npair % 4 == 2):
                        fill.pop(0)()
                    npair += 1
                # stage ctx to SBUF immediately: frees the PSUM bank within
                # one DVE copy so the cx pool gets away with a single buffer
                nc.vector.tensor_copy(out=cxs_e[:, so : so + SLOT], in_=cx_e)
                nc.vector.tensor_copy(out=cxs_o[:, so : so + SLOT], in_=cx_o)
                nc.vector.tensor_copy(out=den2[0:1, so : so + SLOT],
                                      in_=cx_e[DK : DK + 1, :])
                nc.vector.tensor_copy(out=den2[DK : DK + 1, so : so + SLOT],
                                      in_=cx_o[DK : DK + 1, :])
            pending.append(make_norm(cxs_e, cxs_o, den2, hp, qs))

        # ---------- schedule: projections + qb0 attention ----------
        for hp in range(HP):
            q_unit(hp, 0)
        kv_slab(0)
        kv_slab(1)
        for hp in range(HP):
            q_unit(hp, 1)
        q_in.release()
        xt01p.release()
        # pt depth 3: exp(n+1) must not wait on mask/ctx of pair n-1 -- with
        # 256-query pairs the depth-2 chain exposed cross-engine sem latency
        pools['at_sb'] = tc.alloc_tile_pool(name="at_sb", bufs=3)
        pools['nm_sb'] = tc.alloc_tile_pool(name="nm_sb", bufs=2)
        # k_unit epilogues go to DVE (on_act=False): ACT is the pacing engine
        # during qb0 attention (exp chain); v_unit copies stay on ACT to keep
        # the DVE (mask multiplies + den staging) from becoming the new pacer
        kv_fill = [(lambda hp=hp, sb=sb: k_unit(hp, sb, on_act=False))
                   for sb in (2, 3) for hp in range(HP)] + \
                  [(lambda kb=kb, nh=nh: v_unit(kb, nh))
                   for kb in range(8, KB) for nh in range(NH)]
        for hp in range(HP):
            attn_iter(hp, 0, kv_fill)
        for fn in kv_fill:
            fn()
        kv_fill.clear()
        kv_in.release()
        proj_ps.release()

        # ---------- qb1 attention with qb0 epilogue woven in ----------
        mid_ow = tc.alloc_tile_pool(name="mid_ow", bufs=1, side="right")
        xres_sb = mid_ow.tile([P, QC, D], F32)
        nc.gpsimd.dma_start(out=xres_sb,
                            in_=xres.rearrange("(c p) n -> p c n", p=P))
        wo_sb = mid_ow.tile([P, DC, D], BF16)
        nc.gpsimd.dma_start(out=wo_sb, in_=wo.rearrange("(c p) n -> p c n", p=P))
        op_ps = tc.alloc_tile_pool(name="op_ps", bufs=1, space="PSUM", side="right")
        tp_ps = tc.alloc_tile_pool(name="tp_ps", bufs=1, space="PSUM", side="right")

        hpre_map = {}

        def op_half(qc, nh):
            def go():
                if qc not in hpre_map:
                    hpre_map[qc] = ln_wk.tile([P, D], F32, tag="hpre",
                                              bufs=3, name=f"hpre_{qc}")
                hpre = hpre_map[qc]
                ps = op_ps.tile([P, NHW], F32, tag="op")
                for c in range(DC):
                    nc.tensor.matmul(
                        ps, lhsT=ctxT[:, c, qc * P : (qc + 1) * P],
                        rhs=wo_sb[:, c, nh * NHW : (nh + 1) * NHW],
                        start=(c == 0), stop=(c == DC - 1),
                    )
                nc.vector.scalar_tensor_tensor(
                    out=hpre[:, nh * NHW : (nh + 1) * NHW], in0=ps,
                    scalar=1.0, in1=xres_sb[:, qc, nh * NHW : (nh + 1) * NHW],
                    op0=OP.mult, op1=OP.add,
                )
            return go

        def ln_pair(qca, qcb):
            # LN1 for two query chunks with ONE batched Sqrt: halves the
            # sqrt<->exp ACT table switches while attention exp is streaming
            def go():
                mvp = ln_wk.tile([P, 2, 2], F32, tag="mvp", bufs=2)
                ln_stats(ln_wk, hpre_map[qca], mvp[:, 0, :])
                ln_stats(ln_wk, hpre_map[qcb], mvp[:, 1, :])
                rstdp = ln_wk.tile([P, 2], F32, tag="rstdp", bufs=2)
                ln_rstd_batch(mvp, rstdp, 2)
                ln_apply(hpre_map.pop(qca), mvp[:, 0, :], rstdp[:, 0:1],
                         g1b, be1b, h_sb[:, qca, :])
                ln_apply(hpre_map.pop(qcb), mvp[:, 1, :], rstdp[:, 1:2],
                         g1b, be1b, h_sb[:, qcb, :])
            return go

        def transp_half(qc, lo):
            # PSUM->SBUF evacuation on DVE, not ACT: these run woven into qb1
            # attention where ACT (exp) is the pacing engine
            def go():
                for c in range(lo, lo + DC // 2):
                    tp = tp_ps.tile([P, P], BF16, tag="tp")
                    nc.tensor.transpose(tp, h_sb[:, qc, c * P : (c + 1) * P],
                                        ident)
                    nc.vector.tensor_copy(out=hT[:, c, qc * P : (qc + 1) * P],
                                          in_=tp)
            return go

        def transp_unit(qc):
            def go():
                for c in range(DC):
                    tp = tp_ps.tile([P, P], BF16, tag="tp")
                    nc.tensor.transpose(tp, h_sb[:, qc, c * P : (c + 1) * P],
                                        ident)
                    nc.scalar.activation(out=hT[:, c, qc * P : (qc + 1) * P],
                                         in_=tp, func=AF.Copy)
            return go

        # only the out-projection halves weave into sp1 attention: LN1 (ACT
        # Sqrt table load + DVE-heavy apply) would head-of-line-block the ACT
        # exp stream, so all LN1+transpose work runs post-attention instead
        fillers = []
        for qc in range(4):
            fillers += [op_half(qc, 0), op_half(qc, 1)]
        for hp in range(HP):
            attn_iter(hp, 1, fillers)
        for fn in pending:
            fn()
        pending.clear()
        for fn in fillers:
            fn()

        pools['nm_sb'].release()
        pools['at_sb'].release()

    # attention inputs freed BEFORE the LN1/transpose epilogue so the 9.4MB
    # FFN weight DMA (below) streams during the epilogue instead of stalling
    # the first f1 matmuls
    attn_in.release()
    w12_in = tc.alloc_tile_pool(name="w12_in", bufs=1)
    w1_sb = w12_in.tile([P, DC, DF], BF16)
    nc.sync.dma_start(out=w1_sb, in_=w1.rearrange("(c p) n -> p c n", p=P))
    w2_sb = w12_in.tile([P, DFC, D], BF16)
    # separate queue: w2 streams in parallel with w1 (f2 needs it later)
    nc.gpsimd.dma_start(out=w2_sb, in_=w2.rearrange("(c p) n -> p c n", p=P))

    # ---------- rest of out-projection + LN1 + transposes ----------
    for qc in range(4, QC):
        op_half(qc, 0)()
        op_half(qc, 1)()
    for qc in (0, 2, 4, 6):
        ln_pair(qc, qc + 1)()
        transp_unit(qc)()
        transp_unit(qc + 1)()

    tp_ps.release()
    op_ps.release()
    mid_ow.release()
    mid_ctx.release()

    # ====== FFN: f1^T = gelu(w1^T h^T + b1); out = LN2(f1g^T w2 + h) ====
    with tc.tile_pool(name="f1_ps", bufs=3, space="PSUM") as f1_ps, \
         tc.tile_pool(name="f2_ps", bufs=3, space="PSUM") as f2_ps, \
         tc.tile_pool(name="f1g_sb", bufs=2) as f1g_sb, \
         tc.tile_pool(name="out_sb", bufs=3) as out_sb:
        for qb in range(QB):
            qs = slice(qb * QBS, (qb + 1) * QBS)
            f1g = f1g_sb.tile([P, DFC, QBS], BF16, tag="f1g")
            for f in range(DFC):
                ps = f1_ps.tile([P, QBS], F32, tag="f1")
                for c in range(DC):
                    nc.tensor.matmul(
                        ps, lhsT=w1_sb[:, c, f * P : (f + 1) * P],
                        rhs=hT[:, c, qs], start=(c == 0), stop=(c == DC - 1),
                    )
                nc.scalar.activation(out=f1g[:, f, :], in_=ps, func=AF.Gelu,
                                     bias=b1p[:, f : f + 1])
            for sq in range(QBS // P):
                qc = qb * (QBS // P) + sq
                ot = out_sb.tile([P, D], F32, tag="ot")
                for nh in range(NH):
                    ps = f2_ps.tile([P, NHW], F32, tag="f2")
                    for f in range(DFC):
                        nc.tensor.matmul(
                            ps, lhsT=f1g[:, f, sq * P : (sq + 1) * P],
                            rhs=w2_sb[:, f, nh * NHW : (nh + 1) * NHW],
                            start=(f == 0), stop=(f == DFC - 1),
                        )
                    nc.vector.scalar_tensor_tensor(
                        out=ot[:, nh * NHW : (nh + 1) * NHW], in0=ps, scalar=1.0,
                        in1=h_sb[:, qc, nh * NHW : (nh + 1) * NHW],
                        op0=OP.mult, op1=OP.add,
                    )
                nc.vector.tensor_tensor(out=ot, in0=ot, in1=b2b, op=OP.add)
                layer_norm(ln_wk, ot, g2b, be2b, ot)
                nc.sync.dma_start(out=out[qc * P : (qc + 1) * P, :], in_=ot)

    w12_in.release()
    ln_wk.release()
    ffn.release()


def build_program():
    nc = bacc.Bacc("TRN2", target_bir_lowering=False, debug=False,
                   enable_asserts=False, num_devices=N_CORES)
    io = {}

    def din(name, shape, dt):
        io[name] = nc.dram_tensor(name, list(shape), dt, kind="ExternalInput").ap()

    din("xT", (D, S), BF16)
    din("xqT", (D, SQ), BF16)
    din("xres", (SQ, D), F32)
    din("maskT", (4 * len(EXTS) * P, SLOT), BF16)
    din("wq", (D, D), BF16)
    din("wk", (D, D), BF16)
    din("wv", (D, D), BF16)
    din("wo", (D, D), BF16)
    din("w1", (D, DF), BF16)
    din("w2", (DF, D), BF16)
    din("bqp", (P, HP), F32)
    din("bkp", (P, HP), F32)
    din("b1p", (P, DFC), F32)
    for n in ["g1r", "be1r", "g2r", "be2r", "b2r"]:
        din(n, (1, D), F32)
    io["out"] = nc.dram_tensor("out", [SQ, D], F32, kind="ExternalOutput").ap()

    with tile.TileContext(nc) as tc:
        with ExitStack() as ctx:
            emit(ctx, tc, io)
    nc.compile()
    return nc


_NC = None


def _get_program():
    global _NC
    if _NC is None:
        _NC = build_program()
    return _NC


def _qrows(half):
    # interleaved 256-row slots: half 0 gets blocks {0,2,4,6}, half 1 {1,3,5,7}
    return np.concatenate(
        [np.arange(s, s + SLOT) for s in range(half * SLOT, S, 2 * SLOT)]
    )


def _pack_mask(mask_b, qr):
    # per slot s: the last 4 key blocks of EXTS[s], transposed to [k, q]
    allow = (~mask_b).astype(BF)
    blocks = []
    for s, ext in enumerate(EXTS):
        qs = qr[s * SLOT : (s + 1) * SLOT]
        for kb in range(ext - 4, ext):
            blocks.append(allow[qs, kb * P : (kb + 1) * P].T)
    return np.ascontiguousarray(np.concatenate(blocks, axis=0))


def shard_inputs(inputs):
    x = np.asarray(inputs["x"], np.float32)
    mask = np.asarray(inputs["mask"], bool)
    w = {k: np.asarray(inputs[k], np.float32) for k in
         ["wq", "bq", "wk", "bk", "wv", "bv", "wo", "bo", "g1", "be1",
          "w1", "b1", "w2", "b2", "g2", "be2"]}

    base = dict(
        wq=np.ascontiguousarray(w["wq"].astype(BF)),
        wk=np.ascontiguousarray(w["wk"].astype(BF)),
        wv=np.ascontiguousarray(w["wv"].astype(BF)),
        wo=np.ascontiguousarray(w["wo"].astype(BF)),
        w1=np.ascontiguousarray(w["w1"].astype(BF)),
        w2=np.ascontiguousarray(w["w2"].astype(BF)),
        bqp=np.ascontiguousarray(w["bq"].reshape(HP, P).T),
        bkp=np.ascontiguousarray(w["bk"].reshape(HP, P).T),
        b1p=np.ascontiguousarray(w["b1"].reshape(DFC, P).T),
        g1r=np.ascontiguousarray(w["g1"].reshape(1, D)),
        be1r=np.ascontiguousarray(w["be1"].reshape(1, D)),
        g2r=np.ascontiguousarray(w["g2"].reshape(1, D)),
        be2r=np.ascontiguousarray(w["be2"].reshape(1, D)),
        b2r=np.ascontiguousarray(w["b2"].reshape(1, D)),
    )
    # bv and bo fold into the residual: ctx@wo + bo + x with v-bias bv adds
    # a constant row bv@wo (softmax rows sum to 1)
    res_const = (w["bo"] + w["bv"] @ w["wo"]).astype(np.float32)

    in_maps = []
    for c in range(N_CORES):
        b, half = divmod(c, 2)
        qr = _qrows(half)
        xb = x[b]
        xq = xb[qr]
        m = dict(base)
        m["xT"] = np.ascontiguousarray(xb.T.astype(BF))
        m["xqT"] = np.ascontiguousarray(xq.T.astype(BF))
        m["xres"] = np.ascontiguousarray(xq + res_const[None, :])
        m["maskT"] = _pack_mask(mask[b], qr)
        in_maps.append(m)
    return in_maps


def gather_outputs(results):
    y = np.empty((B, S, D), np.float32)
    for c in range(N_CORES):
        b, half = divmod(c, 2)
        y[b, _qrows(half)] = results[c]["out"]
    return y


def kernel(**inputs):
    nc = _get_program()
    in_maps = shard_inputs(inputs)
    res = run_bass_kernel_spmd(nc, in_maps, list(range(N_CORES)))
    return gather_outputs(res.results)


if __name__ == "__main__":
    build_program()
    print("program built ok")



# revision 34
# speedup vs baseline: 1.0064x; 1.0064x over previous
"""Trainium2 Bass kernel for a dense transformer decoder block.

Reference computation (B=4, S=2048, D=768, H=12, DK=64, DF=3072):
    q,k,v = x@wq+bq, x@wk+bk, x@wv+bv          (per-head split, DK=64)
    attn  = softmax(mask(q k^T / 8))
    ctx   = attn @ v
    h     = LN(ctx@wo + bo + x; g1, be1)
    out   = LN(gelu_exact(h@w1 + b1)@w2 + b2 + h; g2, be2)

Sharding: pure data parallel, zero collectives. 8 cores = 4 batch elements
x 2 query groups of 1024 rows. Queries are interleaved at 256-row slot
granularity so the SPMD-uniform per-slot key extents (4, 8, 12, 16) x 128
cover both cores' causal needs with only 20 (vs 24) key-block units of
attention work per core; only the last 4 blocks of each slot's extent ever
need the data mask.
Core 2b+0: 256-row slots {0, 2, 4, 6} of batch b.
Core 2b+1: 256-row slots {1, 3, 5, 7} of batch b.
Every core runs the identical SPMD program; per-core behavior differs only
through input data (sliced/transposed/cast on the host).

Schedule: attention is ACT-(exp)-bound, so independent PE work is woven
between attention iterations to keep the tensor engine dense (and its HAM
clock warm): the sb2/sb3 K,V projections run under qb0 attention, and the
qb0 out-projection + LN1 + h-transposes run under qb1 attention.
"""

from contextlib import ExitStack

import numpy as np
import ml_dtypes

import concourse.bass as bass
import concourse.tile as tile
from concourse import bacc, mybir
from concourse.bass_utils import run_bass_kernel_spmd
from concourse.masks import make_identity

F32 = mybir.dt.float32
BF16 = mybir.dt.bfloat16
AF = mybir.ActivationFunctionType
OP = mybir.AluOpType
BF = ml_dtypes.bfloat16

B, S, D, H, DK = 4, 2048, 768, 12, 64
DF = 4 * D
EPS = 1e-5
P = 128
SQ = 1024            # query rows per core
HP = H // 2          # 6 head pairs
KB = S // P          # 16 key blocks
QB = 2               # query slot-pairs of 512 per core
QBS = 512
SLOT = 256           # attention query slot: 4 slots of 256 rows per core
EXTS = [4, 8, 12, 16]  # key-block extent per slot (block-causal skip)
# Interleaved slot assignment: core 2b+0 gets 256-row blocks {0,2,4,6} of
# batch b, core 2b+1 gets {1,3,5,7}. Slot s on either core then needs at
# most EXTS[s] key blocks, cutting SPMD-uniform attention volume from 24
# key-block-x-512q units to 20 per core, with only the last 4 blocks of
# each slot's extent ever needing the data mask.
DC = D // P          # 6 chunks of the model dim
DFC = DF // P        # 24 chunks of the FFN dim
QC = SQ // P         # 8 query chunks of 128
NH = 2               # 384-wide halves of D for PSUM-friendly matmul N
NHW = D // NH        # 384
SB = S // QBS        # 4 key column slabs

N_CORES = 8


def emit(ctx: ExitStack, tc: tile.TileContext, io: dict):
    nc = tc.nc

    xT, xqT, xres, maskT = io["xT"], io["xqT"], io["xres"], io["maskT"]
    wq, wk, wv, wo, w1, w2 = io["wq"], io["wk"], io["wv"], io["wo"], io["w1"], io["w2"]
    out = io["out"]

    # ---- constants ----------------------------------------------------
    const = ctx.enter_context(tc.tile_pool(name="const", bufs=1))
    ident = const.tile([P, P], BF16)
    make_identity(nc, ident)
    eps_t = const.tile([P, 1], F32)
    nc.vector.memset(eps_t, EPS)

    bqp = const.tile([P, HP], F32)
    nc.gpsimd.dma_start(out=bqp, in_=io["bqp"])
    bkp = const.tile([P, HP], F32)
    nc.gpsimd.dma_start(out=bkp, in_=io["bkp"])
    b1p = const.tile([P, DFC], F32)
    nc.gpsimd.dma_start(out=b1p, in_=io["b1p"])

    def brow(name):
        # [1, D] dram tensor broadcast-DMA'd across 128 partitions
        t = const.tile([P, D], F32, tag=name)
        a = io[name]
        src = bass.AP(tensor=a.tensor, offset=a.offset, ap=[[0, P]] + list(a.ap[1:]))
        nc.gpsimd.dma_start(out=t, in_=src)
        return t

    g1b, be1b, g2b, be2b, b2b = map(brow, ["g1r", "be1r", "g2r", "be2r", "b2r"])

    # ---- FFN-phase tensors: left stack, below attn_in so release order
    # stays LIFO (h/hT are written during the attention epilogue fillers)
    ffn = tc.alloc_tile_pool(name="ffn", bufs=1)
    h_sb = ffn.tile([P, QC, D], BF16)     # LN1 out (residual + FFN rhs)
    hT = ffn.tile([P, DC, SQ], BF16)
    ln_wk = tc.alloc_tile_pool(name="ln_wk", bufs=1)

    # ---- attention inputs (live through attention) --------------------
    attn_in = tc.alloc_tile_pool(name="attn_in", bufs=1)
    KT = attn_in.tile([P, HP, S], BF16)            # K^T, head pairs on partitions
    Vaug = attn_in.tile([P, KB, H, DK + 1], BF16)  # V + ones column per head
    QT = attn_in.tile([P, HP, SQ], BF16)
    # host-packed mask: for each slot s, its last 4 extent blocks
    # [EXTS[s]-4, EXTS[s]) -- the only blocks any core ever masks
    mTs = attn_in.tile([P, 4 * len(EXTS), SLOT], BF16)
    mr = maskT.rearrange("(m p) q -> p m q", p=P)
    nc.gpsimd.dma_start(out=mTs, in_=mr)
    nc.vector.memset(Vaug[:, :, :, DK : DK + 1], 1.0)

    # ---- post-attention inputs (right-side stack, phase-scoped) -------
    mid_ctx = tc.alloc_tile_pool(name="mid_ctx", bufs=1, side="right")
    ctxT = mid_ctx.tile([P, DC, SQ], BF16)

    kv_in = tc.alloc_tile_pool(name="kv_in", bufs=1, side="right")
    wk_sb = kv_in.tile([P, DC, D], BF16)
    wv_sb = kv_in.tile([P, DC, D], BF16)
    xT23 = kv_in.tile([P, DC, S // 2], BF16)
    xt01p = tc.alloc_tile_pool(name="xt01p", bufs=1, side="right")
    xT01 = xt01p.tile([P, DC, S // 2], BF16)
    xTr = xT.rearrange("(c p) s -> p c s", p=P)

    def xT_at(sb):
        t = xT01 if sb < 2 else xT23
        return t, (sb % 2) * QBS

    q_in = tc.alloc_tile_pool(name="q_in", bufs=1, side="right")
    wq_sb = q_in.tile([P, DC, D], BF16)
    xqT_sb = q_in.tile([P, DC, SQ], BF16)
    # split wq per head-pair column so q_unit(hp, 0) can start as soon as its
    # own slice + the first xqT slab land (instead of gating on the full 1.2MB)
    wqr = wq.rearrange("(c p) n -> p c n", p=P)
    for hp in range(HP):
        nc.sync.dma_start(out=wq_sb[:, :, hp * P : (hp + 1) * P],
                          in_=wqr[:, :, hp * P : (hp + 1) * P])
    xqr = xqT.rearrange("(c p) s -> p c s", p=P)
    for sb in range(2):
        nc.scalar.dma_start(out=xqT_sb[:, :, sb * QBS : (sb + 1) * QBS],
                            in_=xqr[:, :, sb * QBS : (sb + 1) * QBS])
    for sb in range(SB):
        dst = xT01 if sb < 2 else xT23
        nc.sync.dma_start(out=dst[:, :, (sb % 2) * QBS : (sb % 2 + 1) * QBS],
                          in_=xTr[:, :, sb * QBS : (sb + 1) * QBS])
    for c in range(DC):
        nc.scalar.dma_start(out=wk_sb[:, c, :],
                            in_=wk.rearrange("(c p) n -> p c n", p=P)[:, c, :])
    for c in range(DC):
        # separate queue from wk so K and V weight loads stream in parallel
        nc.gpsimd.dma_start(out=wv_sb[:, c, :],
                            in_=wv.rearrange("(c p) n -> p c n", p=P)[:, c, :])

    def ln_stats(wk_pool, src, mv_out):
        # DVE-only first half of LN: mean/var of fp32 src [128, 768] -> mv_out
        stats = wk_pool.tile([P, 3, 6], F32, tag="stats", bufs=3)
        for j in range(3):
            nc.vector.bn_stats(out=stats[:, j, :], in_=src[:, j * 256 : (j + 1) * 256])
        nc.vector.bn_aggr(out=mv_out, in_=stats)

    def ln_rstd_batch(mv_all, rstd_all, n):
        # one ACT Sqrt for n LN instances at once: Sqrt lives in a different
        # ACT table set than Exp, so batching keeps the table thrash off the
        # attention phase's exp-saturated ACT queue
        std_n = rstd_all  # in-place staging: sqrt then approx-reciprocal
        nc.scalar.activation(out=std_n[:, 0:n], in_=mv_all[:, 0:n, 1],
                             func=AF.Sqrt, bias=eps_t[:, 0:1])
        nc.vector.reciprocal_approx_fast(out=rstd_all[:, 0:n], in_=std_n[:, 0:n])

    def ln_apply(src, mv, rstd, gb, bb, dst):
        # second half of LN in two fused DVE ops:
        # t = (src - mu) * g;  dst = t * rstd + b
        nc.vector.scalar_tensor_tensor(out=src, in0=src, scalar=mv[:, 0:1],
                                       in1=gb, op0=OP.subtract, op1=OP.mult)
        nc.vector.scalar_tensor_tensor(out=dst, in0=src, scalar=rstd,
                                       in1=bb, op0=OP.mult, op1=OP.add)

    def layer_norm(wk_pool, src, gb, bb, dst):
        # full inline LN for the FFN epilogue (off the exp-critical phase)
        mv = wk_pool.tile([P, 1, 2], F32, tag="mv", bufs=3)
        ln_stats(wk_pool, src, mv[:, 0, :])
        rstd = wk_pool.tile([P, 1], F32, tag="rstd", bufs=3)
        ln_rstd_batch(mv, rstd, 1)
        ln_apply(src, mv[:, 0, :], rstd[:, 0:1], gb, bb, dst)

    proj_ps = tc.alloc_tile_pool(name="proj_ps", bufs=2, space="PSUM", side="right")
    with tc.tile_pool(name="sc_ps", bufs=2, space="PSUM") as sc_ps, \
         tc.tile_pool(name="cx_ps", bufs=1, space="PSUM") as cx_ps:
        # at_sb/nm_sb are allocated only once the q-projection inputs are
        # released -- their SBUF footprints must not overlap
        pools = {}

        # ---------- projection work units ----------
        def q_unit(hp, sb):
            ps = proj_ps.tile([P, QBS], F32, tag="proj")
            for c in range(DC):
                nc.tensor.matmul(
                    ps, lhsT=wq_sb[:, c, hp * P : (hp + 1) * P],
                    rhs=xqT_sb[:, c, sb * QBS : (sb + 1) * QBS],
                    start=(c == 0), stop=(c == DC - 1),
                )
            nc.scalar.activation(
                out=QT[:, hp, sb * QBS : (sb + 1) * QBS], in_=ps,
                func=AF.Identity, bias=bqp[:, hp : hp + 1],
            )

        def k_unit(hp, sb, on_act=True):
            xt, off = xT_at(sb)
            ps = proj_ps.tile([P, QBS], F32, tag="proj")
            for c in range(DC):
                nc.tensor.matmul(
                    ps, lhsT=wk_sb[:, c, hp * P : (hp + 1) * P],
                    rhs=xt[:, c, off : off + QBS],
                    start=(c == 0), stop=(c == DC - 1),
                )
            if on_act:
                nc.scalar.activation(
                    out=KT[:, hp, sb * QBS : (sb + 1) * QBS], in_=ps,
                    func=AF.Identity, bias=bkp[:, hp : hp + 1],
                )
            else:
                # inside the attention interleave ACT is the bottleneck chain
                nc.vector.tensor_scalar_add(
                    out=KT[:, hp, sb * QBS : (sb + 1) * QBS], in0=ps,
                    scalar1=bkp[:, hp : hp + 1],
                )

        def v_unit(kb, nh, on_act=True):
            xt, off = xT_at(kb // (QBS // P))
            kb_off = off // P + kb % (QBS // P)
            ps = proj_ps.tile([P, QBS], F32, tag="proj")
            psv = ps[:, 0:NHW]
            for c in range(DC):
                nc.tensor.matmul(
                    psv, lhsT=xt[:, c, kb_off * P : (kb_off + 1) * P],
                    rhs=wv_sb[:, c, nh * NHW : (nh + 1) * NHW],
                    start=(c == 0), stop=(c == DC - 1),
                )
            if on_act:
                nc.scalar.activation(
                    out=Vaug[:, kb, nh * 6 : (nh + 1) * 6, 0:DK],
                    in_=psv.rearrange("p (h d) -> p h d", d=DK),
                    func=AF.Copy,
                )
            else:
                nc.vector.tensor_copy(
                    out=Vaug[:, kb, nh * 6 : (nh + 1) * 6, 0:DK],
                    in_=psv.rearrange("p (h d) -> p h d", d=DK),
                )

        def kv_slab(sb, on_act=True):
            for hp in range(HP):
                k_unit(hp, sb, on_act)
            for j in range(QBS // P):
                for nh in range(NH):
                    v_unit(sb * (QBS // P) + j, nh, on_act)

        # ---------- attention iteration ----------
        pending = []

        def make_norm(cxs_e, cxs_o, den2, hp, qs):
            def go():
                # one reciprocal serves both heads: its cost scales with the
                # free size, not the partition count; dens are in (0, 2048] so
                # the ~51-ULP approx is safe and ~5x faster than the iterative
                # divide
                rec2 = pools['nm_sb'].tile([DK + 1, QBS], F32, tag="rec2", bufs=1)
                nc.vector.reciprocal_approx_fast(out=rec2, in_=den2)
                # partition_broadcast replicates the tile's physical partition
                # 0, so the head-odd reciprocal must move to its own base-0
                # tile first
                rec_o = pools['nm_sb'].tile([1, QBS], F32, tag="rec_o", bufs=1)
                nc.vector.tensor_copy(out=rec_o, in_=rec2[DK : DK + 1, :])
                for i, (cxs, pb) in enumerate(((cxs_e, 0), (cxs_o, DK))):
                    src_r = rec2[0:1, :] if i == 0 else rec_o[0:1, :]
                    den_b = pools['nm_sb'].tile([DK, QBS], F32, tag="den_b", bufs=1)
                    nc.gpsimd.partition_broadcast(den_b, src_r)
                    nc.vector.tensor_tensor(
                        out=ctxT[pb : pb + DK, hp, qs], in0=cxs[0:DK, :],
                        in1=den_b, op=OP.mult,
                    )
            return go

        def attn_iter(hp, sp, fill=None):
            # one slot-pair: slots (2*sp, 2*sp+1), 256 queries each; the
            # normalize machinery operates on the combined 512-query span
            qs = slice(sp * QBS, (sp + 1) * QBS)
            cxs_e = pools['nm_sb'].tile([DK + 1, QBS], F32, tag="cxs_e")
            cxs_o = pools['nm_sb'].tile([DK + 1, QBS], F32, tag="cxs_o")
            den2 = pools['nm_sb'].tile([DK + 1, QBS], F32, tag="den2")
            nc.vector.memset(den2, 1.0)
            npair = 0
            for si in range(2):
                s = 2 * sp + si
                ext = EXTS[s]
                sq = slice(s * SLOT, (s + 1) * SLOT)
                so = si * SLOT
                cx_e = cx_ps.tile([DK + 1, SLOT], F32, tag="cx_e")
                cx_o = cx_ps.tile([DK + 1, SLOT], F32, tag="cx_o")
                for gb in range(0, ext, 2):
                    pt = pools['at_sb'].tile([P, 2, 2, SLOT], BF16, tag="pt")
                    sc = sc_ps.tile([P, 2, 2, SLOT], F32, tag="sc")
                    for gi in range(2):
                        g = gb + gi
                        ks = slice(g * P, (g + 1) * P)
                        # the two heads of a pair hit disjoint PE row groups
                        # and run concurrently in the array
                        nc.tensor.matmul(sc[:, 0, gi, :], lhsT=KT[0:DK, hp, ks],
                                         rhs=QT[0:DK, hp, sq],
                                         start=True, stop=True)
                        nc.tensor.matmul(sc[:, 1, gi, :], lhsT=KT[DK:P, hp, ks],
                                         rhs=QT[DK:P, hp, sq],
                                         start=True, stop=True)
                    # one exp per g-pair (1024 elems) keeps the per-ACTIVATE
                    # 352-cycle overhead amortized despite the smaller slots
                    nc.scalar.activation(out=pt, in_=sc, func=AF.Exp,
                                         scale=1.0 / 8.0)
                    if gb >= ext - 4:
                        mi = 4 * s + gb - (ext - 4)
                        mq = mTs[:, mi : mi + 2, :]
                        for hh in range(2):
                            nc.vector.tensor_tensor(
                                out=pt[:, hh, :, :], in0=pt[:, hh, :, :],
                                in1=mq, op=OP.mult,
                            )
                    for gi in range(2):
                        g = gb + gi
                        nc.tensor.matmul(cx_e, lhsT=Vaug[:, g, 2 * hp, :],
                                         rhs=pt[:, 0, gi, :],
                                         start=(g == 0), stop=(g == ext - 1))
                        nc.tensor.matmul(cx_o, lhsT=Vaug[:, g, 2 * hp + 1, :],
                                         rhs=pt[:, 1, gi, :],
                                         start=(g == 0), stop=(g == ext - 1))
                    if si == 0 and gb == 2 and pending:
                        # previous iteration's normalize: emitted after this
                        # iteration's first blocks so the DVE reciprocal
                        # never delays the mask multiplies
                        pending.pop()()
                    # the PE is in-order: filler matmuls only absorb the
                    # exp-wait bubbles if woven BETWEEN key-block groups
                    if fill and (sp == 0 or npair % 4 == 2):
                        fill.pop(0)()
                    npair += 1
                # stage ctx to SBUF immediately: frees the PSUM bank within
                # one DVE copy so the cx pool gets away with a single buffer
                nc.vector.tensor_copy(out=cxs_e[:, so : so + SLOT], in_=cx_e)
                nc.vector.tensor_copy(out=cxs_o[:, so : so + SLOT], in_=cx_o)
                nc.vector.tensor_copy(out=den2[0:1, so : so + SLOT],
                                      in_=cx_e[DK : DK + 1, :])
                nc.vector.tensor_copy(out=den2[DK : DK + 1, so : so + SLOT],
                                      in_=cx_o[DK : DK + 1, :])
            pending.append(make_norm(cxs_e, cxs_o, den2, hp, qs))

        # ---------- schedule: projections + qb0 attention ----------
        for hp in range(HP):
            q_unit(hp, 0)
        kv_slab(0)
        kv_slab(1)
        for hp in range(HP):
            q_unit(hp, 1)
        q_in.release()
        xt01p.release()
        # wo + streamed xres chunks live on the left stack from sp0 on: no
        # SBUF pool swap (and thus no scheduling barrier) between the sp0
        # and sp1 attention phases
        wo_x = tc.alloc_tile_pool(name="wo_x", bufs=1)
        wo_sb = wo_x.tile([P, DC, D], BF16)
        nc.gpsimd.dma_start(out=wo_sb, in_=wo.rearrange("(c p) n -> p c n", p=P))
        xrr = xres.rearrange("(c p) n -> p c n", p=P)
        # pt depth 3: exp(n+1) must not wait on mask/ctx of pair n-1 -- with
        # 256-query pairs the depth-2 chain exposed cross-engine sem latency
        pools['at_sb'] = tc.alloc_tile_pool(name="at_sb", bufs=3)
        pools['nm_sb'] = tc.alloc_tile_pool(name="nm_sb", bufs=2)
        # k_unit epilogues go to DVE (on_act=False): ACT is the pacing engine
        # during qb0 attention (exp chain); v_unit copies stay on ACT to keep
        # the DVE (mask multiplies + den staging) from becoming the new pacer
        kv_fill = [(lambda hp=hp, sb=sb: k_unit(hp, sb, on_act=False))
                   for sb in (2, 3) for hp in range(HP)] + \
                  [(lambda kb=kb, nh=nh: v_unit(kb, nh))
                   for kb in range(8, KB) for nh in range(NH)]
        for hp in range(HP):
            attn_iter(hp, 0, kv_fill)
        for fn in kv_fill:
            fn()
        kv_fill.clear()
        kv_in.release()
        proj_ps.release()

        # ---------- qb1 attention with qb0 epilogue woven in ----------
        op_ps = tc.alloc_tile_pool(name="op_ps", bufs=1, space="PSUM", side="right")
        tp_ps = tc.alloc_tile_pool(name="tp_ps", bufs=1, space="PSUM", side="right")

        hpre_map = {}
        xres_map = {}

        def xres_fetch(qc):
            # residual chunks stream through a 3-slot rotation on the
            # otherwise-idle sync queue (3KB each vs a 3MB monolithic load)
            xres_map[qc] = wo_x.tile([P, D], F32, tag="xres", bufs=3,
                                     name=f"xres_{qc}")
            nc.sync.dma_start(out=xres_map[qc], in_=xrr[:, qc, :])

        def op_half(qc, nh):
            def go():
                if qc not in hpre_map:
                    hpre_map[qc] = ln_wk.tile([P, D], F32, tag="hpre",
                                              bufs=3, name=f"hpre_{qc}")
                hpre = hpre_map[qc]
                ps = op_ps.tile([P, NHW], F32, tag="op")
                for c in range(DC):
                    nc.tensor.matmul(
                        ps, lhsT=ctxT[:, c, qc * P : (qc + 1) * P],
                        rhs=wo_sb[:, c, nh * NHW : (nh + 1) * NHW],
                        start=(c == 0), stop=(c == DC - 1),
                    )
                nc.vector.scalar_tensor_tensor(
                    out=hpre[:, nh * NHW : (nh + 1) * NHW], in0=ps,
                    scalar=1.0,
                    in1=xres_map[qc][:, nh * NHW : (nh + 1) * NHW],
                    op0=OP.mult, op1=OP.add,
                )
                if nh == 1:
                    xres_map.pop(qc)
                    if qc + 3 < QC:
                        xres_fetch(qc + 3)
            return go

        def ln_pair(qca, qcb):
            # LN1 for two query chunks with ONE batched Sqrt: halves the
            # sqrt<->exp ACT table switches while attention exp is streaming
            def go():
                mvp = ln_wk.tile([P, 2, 2], F32, tag="mvp", bufs=2)
                ln_stats(ln_wk, hpre_map[qca], mvp[:, 0, :])
                ln_stats(ln_wk, hpre_map[qcb], mvp[:, 1, :])
                rstdp = ln_wk.tile([P, 2], F32, tag="rstdp", bufs=2)
                ln_rstd_batch(mvp, rstdp, 2)
                ln_apply(hpre_map.pop(qca), mvp[:, 0, :], rstdp[:, 0:1],
                         g1b, be1b, h_sb[:, qca, :])
                ln_apply(hpre_map.pop(qcb), mvp[:, 1, :], rstdp[:, 1:2],
                         g1b, be1b, h_sb[:, qcb, :])
            return go

        def transp_half(qc, lo):
            # PSUM->SBUF evacuation on DVE, not ACT: these run woven into qb1
            # attention where ACT (exp) is the pacing engine
            def go():
                for c in range(lo, lo + DC // 2):
                    tp = tp_ps.tile([P, P], BF16, tag="tp")
                    nc.tensor.transpose(tp, h_sb[:, qc, c * P : (c + 1) * P],
                                        ident)
                    nc.vector.tensor_copy(out=hT[:, c, qc * P : (qc + 1) * P],
                                          in_=tp)
            return go

        def transp_unit(qc):
            def go():
                for c in range(DC):
                    tp = tp_ps.tile([P, P], BF16, tag="tp")
                    nc.tensor.transpose(tp, h_sb[:, qc, c * P : (c + 1) * P],
                                        ident)
                    nc.scalar.activation(out=hT[:, c, qc * P : (qc + 1) * P],
                                         in_=tp, func=AF.Copy)
            return go

        # only the out-projection halves weave into sp1 attention: LN1 (ACT
        # Sqrt table load + DVE-heavy apply) would head-of-line-block the ACT
        # exp stream, so all LN1+transpose work runs post-attention instead
        for qc in range(3):
            xres_fetch(qc)
        fillers = []
        for qc in range(4):
            fillers += [op_half(qc, 0), op_half(qc, 1)]
        for hp in range(HP):
            attn_iter(hp, 1, fillers)
        for fn in pending:
            fn()
        pending.clear()
        for fn in fillers:
            fn()

        # ---------- rest of out-projection + LN1 + transposes ----------
        for qc in range(4, QC):
            op_half(qc, 0)()
            op_half(qc, 1)()
        for qc in (0, 2, 4, 6):
            ln_pair(qc, qc + 1)()
            transp_unit(qc)()
            transp_unit(qc + 1)()
        pools['nm_sb'].release()
        pools['at_sb'].release()

    wo_x.release()
    attn_in.release()
    tp_ps.release()
    op_ps.release()
    mid_ctx.release()

    # ====== FFN: f1^T = gelu(w1^T h^T + b1); out = LN2(f1g^T w2 + h) ====
    with tc.tile_pool(name="w12_in", bufs=1) as w12_in, \
         tc.tile_pool(name="f1_ps", bufs=3, space="PSUM") as f1_ps, \
         tc.tile_pool(name="f2_ps", bufs=3, space="PSUM") as f2_ps, \
         tc.tile_pool(name="f1g_sb", bufs=2) as f1g_sb, \
         tc.tile_pool(name="out_sb", bufs=3) as out_sb:
        w1_sb = w12_in.tile([P, DC, DF], BF16)
        nc.sync.dma_start(out=w1_sb, in_=w1.rearrange("(c p) n -> p c n", p=P))
        w2_sb = w12_in.tile([P, DFC, D], BF16)
        # separate queue: w2 streams in parallel with w1 (f2 needs it later)
        nc.gpsimd.dma_start(out=w2_sb, in_=w2.rearrange("(c p) n -> p c n", p=P))
        for qb in range(QB):
            qs = slice(qb * QBS, (qb + 1) * QBS)
            f1g = f1g_sb.tile([P, DFC, QBS], BF16, tag="f1g")
            for f in range(DFC):
                ps = f1_ps.tile([P, QBS], F32, tag="f1")
                for c in range(DC):
                    nc.tensor.matmul(
                        ps, lhsT=w1_sb[:, c, f * P : (f + 1) * P],
                        rhs=hT[:, c, qs], start=(c == 0), stop=(c == DC - 1),
                    )
                nc.scalar.activation(out=f1g[:, f, :], in_=ps, func=AF.Gelu,
                                     bias=b1p[:, f : f + 1])
            for sq in range(QBS // P):
                qc = qb * (QBS // P) + sq
                ot = out_sb.tile([P, D], F32, tag="ot")
                for nh in range(NH):
                    ps = f2_ps.tile([P, NHW], F32, tag="f2")
                    for f in range(DFC):
                        nc.tensor.matmul(
                            ps, lhsT=f1g[:, f, sq * P : (sq + 1) * P],
                            rhs=w2_sb[:, f, nh * NHW : (nh + 1) * NHW],
                            start=(f == 0), stop=(f == DFC - 1),
                        )
                    nc.vector.scalar_tensor_tensor(
                        out=ot[:, nh * NHW : (nh + 1) * NHW], in0=ps, scalar=1.0,
                        in1=h_sb[:, qc, nh * NHW : (nh + 1) * NHW],
                        op0=OP.mult, op1=OP.add,
                    )
                nc.vector.tensor_tensor(out=ot, in0=ot, in1=b2b, op=OP.add)
                layer_norm(ln_wk, ot, g2b, be2b, ot)
                nc.sync.dma_start(out=out[qc * P : (qc + 1) * P, :], in_=ot)

    ln_wk.release()
    ffn.release()


def build_program():
    nc = bacc.Bacc("TRN2", target_bir_lowering=False, debug=False,
                   enable_asserts=False, num_devices=N_CORES)
    io = {}

    def din(name, shape, dt):
        io[name] = nc.dram_tensor(name, list(shape), dt, kind="ExternalInput").ap()

    din("xT", (D, S), BF16)
    din("xqT", (D, SQ), BF16)
    din("xres", (SQ, D), F32)
    din("maskT", (4 * len(EXTS) * P, SLOT), BF16)
    din("wq", (D, D), BF16)
    din("wk", (D, D), BF16)
    din("wv", (D, D), BF16)
    din("wo", (D, D), BF16)
    din("w1", (D, DF), BF16)
    din("w2", (DF, D), BF16)
    din("bqp", (P, HP), F32)
    din("bkp", (P, HP), F32)
    din("b1p", (P, DFC), F32)
    for n in ["g1r", "be1r", "g2r", "be2r", "b2r"]:
        din(n, (1, D), F32)
    io["out"] = nc.dram_tensor("out", [SQ, D], F32, kind="ExternalOutput").ap()

    with tile.TileContext(nc) as tc:
        with ExitStack() as ctx:
            emit(ctx, tc, io)
    nc.compile()
    return nc


_NC = None


def _get_program():
    global _NC
    if _NC is None:
        _NC = build_program()
    return _NC


def _qrows(half):
    # interleaved 256-row slots: half 0 gets blocks {0,2,4,6}, half 1 {1,3,5,7}
    return np.concatenate(
        [np.arange(s, s + SLOT) for s in range(half * SLOT, S, 2 * SLOT)]
    )


def _pack_mask(mask_b, qr):
    # per slot s: the last 4 key blocks of EXTS[s], transposed to [k, q]
    allow = (~mask_b).astype(BF)
    blocks = []
    for s, ext in enumerate(EXTS):
        qs = qr[s * SLOT : (s + 1) * SLOT]
        for kb in range(ext - 4, ext):
            blocks.append(allow[qs, kb * P : (kb + 1) * P].T)
    return np.ascontiguousarray(np.concatenate(blocks, axis=0))


def shard_inputs(inputs):
    x = np.asarray(inputs["x"], np.float32)
    mask = np.asarray(inputs["mask"], bool)
    w = {k: np.asarray(inputs[k], np.float32) for k in
         ["wq", "bq", "wk", "bk", "wv", "bv", "wo", "bo", "g1", "be1",
          "w1", "b1", "w2", "b2", "g2", "be2"]}

    base = dict(
        wq=np.ascontiguousarray(w["wq"].astype(BF)),
        wk=np.ascontiguousarray(w["wk"].astype(BF)),
        wv=np.ascontiguousarray(w["wv"].astype(BF)),
        wo=np.ascontiguousarray(w["wo"].astype(BF)),
        w1=np.ascontiguousarray(w["w1"].astype(BF)),
        w2=np.ascontiguousarray(w["w2"].astype(BF)),
        bqp=np.ascontiguousarray(w["bq"].reshape(HP, P).T),
        bkp=np.ascontiguousarray(w["bk"].reshape(HP, P).T),
        b1p=np.ascontiguousarray(w["b1"].reshape(DFC, P).T),
        g1r=np.ascontiguousarray(w["g1"].reshape(1, D)),
        be1r=np.ascontiguousarray(w["be1"].reshape(1, D)),
        g2r=np.ascontiguousarray(w["g2"].reshape(1, D)),
        be2r=np.ascontiguousarray(w["be2"].reshape(1, D)),
        b2r=np.ascontiguousarray(w["b2"].reshape(1, D)),
    )
    # bv and bo fold into the residual: ctx@wo + bo + x with v-bias bv adds
    # a constant row bv@wo (softmax rows sum to 1)
    res_const = (w["bo"] + w["bv"] @ w["wo"]).astype(np.float32)

    in_maps = []
    for c in range(N_CORES):
        b, half = divmod(c, 2)
        qr = _qrows(half)
        xb = x[b]
        xq = xb[qr]
        m = dict(base)
        m["xT"] = np.ascontiguousarray(xb.T.astype(BF))
        m["xqT"] = np.ascontiguousarray(xq.T.astype(BF))
        m["xres"] = np.ascontiguousarray(xq + res_const[None, :])
        m["maskT"] = _pack_mask(mask[b], qr)
        in_maps.append(m)
    return in_maps


def gather_outputs(results):
    y = np.empty((B, S, D), np.float32)
    for c in range(N_CORES):
        b, half = divmod(c, 2)
        y[b, _qrows(half)] = results[c]["out"]
    return y


def kernel(**inputs):
    nc = _get_program()
    in_maps = shard_inputs(inputs)
    res = run_bass_kernel_spmd(nc, in_maps, list(range(N_CORES)))
    return gather_outputs(res.results)


if __name__ == "__main__":
    build_program()
    print("program built ok")

